# revision 1
# baseline (speedup 1.0000x reference)
"""Trainium2 Bass kernel for LlamaAttention (B=1, S=2048, HID=2048, H=32, KV=8, D=64).

Sharding (8 cores): tensor-parallel over heads. Core c owns q-heads 4c..4c+3 and
kv-head c. Each core computes QKV projections for its heads, RoPE, causal
attention, then the per-core attention outputs [256, S] are AllGathered and each
core computes 256 output features of o_proj (Wo row-sharded). Host concatenates
the 8 column shards and transposes.

All large matmuls run as float32r (fp32 data, fp22 multiply, fp32 accumulate).
"""

import numpy as np

import concourse.bass as bass
import concourse.mybir as mybir
import concourse.tile as tile
from concourse import bacc
from concourse import bass_utils
from concourse.bass_interp import get_hw_module
from concourse.masks import make_identity

S = 2048
HID = 2048
H = 32
KV = 8
D = 64
NCORES = 8
HQ = H // NCORES          # 4 q heads per core
BASE = 10000.0
F32 = mybir.dt.float32
F32R = mybir.dt.float32r
AF = mybir.ActivationFunctionType
ST = S // 512             # 4 s/q tiles of 512
KO = HID // 128           # 16 contraction chunks
NEG = -1.0e30
HALF_PI = float(np.pi / 2)
LN_BASE = float(np.log(BASE))

# run device-side RoPE table computation (sigmoid/exp/sin on ACT). If False,
# cos/sin tables are computed on host and passed as inputs.
DEVICE_TABLES = True
TIMING_STUB = False


def r(ap):
    return ap.bitcast(F32R)


def build_body(tc, aps):
    nc = tc.nc
    hiddenT = aps["hiddenT"]
    wqkvT = aps["wqkvT"]
    woT = aps["woT"]
    trimask = aps["trimask"]
    outT = aps["outT"]

    hT3 = hiddenT.rearrange("(ko p) s -> p ko s", p=128)
    wq3 = wqkvT.rearrange("(ko p) m -> p ko m", p=128)
    wo3 = woT.rearrange("(ko p) m -> p ko m", p=128)

    from contextlib import ExitStack
    es = ExitStack()
    const_pool = es.enter_context(tc.tile_pool(name="const", bufs=1))
    qkv_pool = es.enter_context(tc.tile_pool(name="qkvout", bufs=1))
    dram = es.enter_context(tc.tile_pool(name="dram", bufs=1, space="DRAM"))

    # ---- constants ----
    mask_sb = const_pool.tile([128, 128], F32, tag="mask")
    nc.sync.dma_start(mask_sb[:], trimask[:])
    ident = const_pool.tile([64, 64], F32, tag="ident")
    make_identity(nc, ident[:])

    # ---- RoPE tables: cosT2/sinT2 [128, S] ----
    cosT2 = const_pool.tile([128, S], F32, tag="cos")
    sinT2 = const_pool.tile([128, S], F32, tag="sin")
    if DEVICE_TABLES:
        posi = aps["posi"]
        powers = aps["powers"]
        with tc.tile_pool(name="tabtmp", bufs=1) as tab:
            # inv_freq = exp(-ln(BASE) * sigmoid(powers))  [32, 1]
            pw = tab.tile([32, 1], F32, tag="pw")
            nc.sync.dma_start(pw[:], powers[:])
            sg = tab.tile([32, 1], F32, tag="sg")
            nc.scalar.activation(sg[:], pw[:], AF.Sigmoid)
            invf = tab.tile([32, 1], F32, tag="invf")
            nc.scalar.activation(invf[:], sg[:], AF.Exp, scale=-LN_BASE)
            # signed replicate to [128, 1]: bands (-f, +f, -f, +f)
            invs = tab.tile([128, 1], F32, tag="invs")
            for b in range(4):
                nc.sync.dma_start(invs[b * 32:(b + 1) * 32, :], invf[:])
            for b in (0, 2):
                nc.scalar.activation(
                    invs[b * 32:(b + 1) * 32, :], invs[b * 32:(b + 1) * 32, :],
                    AF.Copy, scale=-1.0)
            # pos as f32 on one partition
            pos_i = tab.tile([1, S], mybir.dt.int32, tag="posi")
            nc.sync.dma_start(pos_i[:], posi[:])
            pos_f = tab.tile([1, S], F32, tag="posf")
            nc.vector.tensor_copy(pos_f[:], pos_i[:])
            pos_b = tab.tile([128, S], F32, tag="posb")
            nc.gpsimd.partition_broadcast(pos_b[:], pos_f[:])
            # freqs_signed = pos * invf_signed ; write into cosT2 temp
            nc.vector.tensor_scalar_mul(cosT2[:], pos_b[:], invs[:, 0:1])
            # range-reduce mod 2*pi so ACT Sin stays in its accurate domain:
            # x -= 2pi * round_to_int(x / 2pi)
            TWO_PI = 6.283185307179586
            u = tab.tile([128, S], F32, tag="u")
            nc.vector.tensor_scalar_mul(u[:], cosT2[:], 1.0 / TWO_PI)
            ui = tab.tile([128, S], mybir.dt.int32, tag="ui")
            nc.vector.tensor_copy(ui[:], u[:])
            nc.vector.tensor_copy(u[:], ui[:])
            nc.vector.tensor_scalar_mul(u[:], u[:], -TWO_PI)
            nc.vector.tensor_add(cosT2[:], cosT2[:], u[:])
            # sin/cos from signed freqs (sin odd, sin(x+pi/2)=cos(x))
            hpi = tab.tile([128, 1], F32, tag="hpi")
            nc.gpsimd.memset(hpi[:], HALF_PI)
            nc.scalar.activation(sinT2[:], cosT2[:], AF.Sin)
            nc.scalar.activation(cosT2[:], cosT2[:], AF.Sin, bias=hpi[:])
    else:
        nc.sync.dma_start(cosT2[:], aps["cos_t"][:])
        nc.sync.dma_start(sinT2[:], aps["sin_t"][:])

    # ---- persistent QKV outputs ----
    qT = [qkv_pool.tile([128, S], F32R, tag=f"qT{p}", name=f"qT{p}") for p in range(2)]
    kT2 = qkv_pool.tile([128, S], F32R, tag="kT2")
    vT = qkv_pool.tile([64, S], F32, tag="vT")
    vones = qkv_pool.tile([128, KO, 65], F32R, tag="vones")
    ones_f = const_pool.tile([128, 1], F32, tag="onesf")
    nc.gpsimd.memset(ones_f[:], 1.0)
    nc.vector.tensor_copy(vones[:, :, 64:65], ones_f[:, 0:1, None].to_broadcast((128, KO, 1)))

    wq_sb = qkv_pool.tile([128, KO, 384], F32R, tag="wq")
    nc.sync.dma_start(wq_sb[:], wq3.bitcast(F32R))

    # ---- QKV projection + RoPE ----
    def rope(ps, dst, n_half, st):
        """ps: psum [64*n_half, 512] raw (qT pair or kT). dst[:, st*512:...] = roped."""
        sl = slice(st * 512, (st + 1) * 512)
        cs = cosT2[0:64 * n_half, sl]
        sn = sinT2[0:64 * n_half, sl]
        craw = tmp_pool.tile([64 * n_half, 512], F32, tag="craw")
        nc.scalar.activation(craw[:], ps[:], AF.Copy)
        sw = tmp_pool.tile([64 * n_half, 512], F32, tag="swap")
        for b in range(n_half):
            nc.sync.dma_start(sw[b * 64:b * 64 + 32, :], craw[b * 64 + 32:b * 64 + 64, :])
            nc.sync.dma_start(sw[b * 64 + 32:b * 64 + 64, :], craw[b * 64:b * 64 + 32, :])
        t1 = tmp_pool.tile([64 * n_half, 512], F32, tag="t1")
        nc.vector.tensor_mul(t1[:], ps[:], cs)
        nc.vector.tensor_mul(sw[:], sw[:], sn)
        nc.vector.tensor_add(dst[0:64 * n_half, sl], t1[:], sw[:])

    with (
        tc.tile_pool(name="hidd", bufs=2) as hidd_pool,
        tc.tile_pool(name="qkvps", bufs=2, space="PSUM") as qkv_ps,
        tc.tile_pool(name="kvps", bufs=2, space="PSUM") as kv_ps,
        tc.tile_pool(name="vtps", bufs=2, space="PSUM") as vt_ps,
        tc.tile_pool(name="ropetmp", bufs=3) as tmp_pool,
    ):
        for st in range(ST):
            ht = hidd_pool.tile([128, KO, 512], F32R, tag="ht")
            nc.sync.dma_start(ht[:], hT3[:, :, st * 512:(st + 1) * 512].bitcast(F32R))
            for mt in range(2):  # q head pairs
                ps = qkv_ps.tile([128, 512], F32, tag="qps")
                for ko in range(KO):
                    nc.tensor.matmul(
                        ps[:], wq_sb[:, ko, mt * 128:(mt + 1) * 128],
                        ht[:, ko, :], start=(ko == 0), stop=(ko == KO - 1))
                rope(ps, qT[mt], 2, st)
            # k (M=64)
            psk = kv_ps.tile([64, 512], F32, tag="kps")
            for ko in range(KO):
                nc.tensor.matmul(psk[:], wq_sb[:, ko, 256:320], ht[:, ko, :],
                                 start=(ko == 0), stop=(ko == KO - 1))
            rope(psk, kT2, 1, st)
            nc.sync.dma_start(kT2[64:128, st * 512:(st + 1) * 512],
                              kT2[0:64, st * 512:(st + 1) * 512])
            # v (M=64)
            psv = kv_ps.tile([64, 512], F32, tag="vps")
            for ko in range(KO):
                nc.tensor.matmul(psv[:], wq_sb[:, ko, 320:384], ht[:, ko, :],
                                 start=(ko == 0), stop=(ko == KO - 1))
            nc.scalar.activation(vT[:, st * 512:(st + 1) * 512], psv[:], AF.Copy)
        # transpose v -> vones [128, ko, 0:64]
        for ki in range(KO):
            pvt = vt_ps.tile([128, 64], F32, tag="vt")
            nc.tensor.transpose(pvt[:], vT[:, ki * 128:(ki + 1) * 128], ident[:])
            nc.vector.tensor_copy(vones[:, ki, 0:64], pvt[:])

    # ---- attention ----
    cc_in = dram.tile([HQ * D, S], F32)
    attn_un = []   # (h, qt) -> unnormalized attnT [64, 512]
    with (
        tc.tile_pool(name="sps", bufs=6, space="PSUM") as s_ps,
        tc.tile_pool(name="aps", bufs=2, space="PSUM") as a_ps,
        tc.tile_pool(name="expp", bufs=24) as exp_pool,
        tc.tile_pool(name="attnun", bufs=16) as un_pool,
        tc.tile_pool(name="sums", bufs=1) as sums_pool,
        tc.tile_pool(name="norm", bufs=2) as norm_pool,
    ):
        sums16 = sums_pool.tile([16, 512], F32, tag="sums16")
        for qt in range(ST):
            nki = 4 * qt + 4
            for hp in range(2):
                pa = [a_ps.tile([65, 512], F32, tag="pattn", name=f"pattn{qt}_{hp}_{i}") for i in range(2)]
                staged = []
                for ki in range(nki):
                    for x in range(2):  # head 2hp+x
                        m = ki - 4 * qt
                        lo = max(0, m) * 128
                        pss = s_ps.tile([128, 512], F32, tag="ps_s")
                        nc.tensor.matmul(
                            pss[:, lo:512],
                            kT2[x * 64:(x + 1) * 64, ki * 128:(ki + 1) * 128],
                            qT[hp][x * 64:(x + 1) * 64, qt * 512 + lo:(qt + 1) * 512],
                            start=True, stop=True)
                        if m >= 0:  # diagonal block: apply triangular causal mask
                            nc.vector.tensor_add(
                                pss[:, m * 128:(m + 1) * 128],
                                pss[:, m * 128:(m + 1) * 128], mask_sb[:])
                        et = exp_pool.tile([128, 512], F32R, tag="expt")
                        nc.scalar.activation(et[:, lo:512], pss[:, lo:512],
                                             AF.Exp, scale=0.125)
                        staged.append((ki, x, lo, et))
                # second pass: attn@v matmuls read staged SBUF exp tiles, so the
                # PE never blocks on ACT latency mid-stream
                for ki, x, lo, et in staged:
                    nc.tensor.matmul(pa[x][:, lo:512], vones[:, ki, :],
                                     et[:, lo:512],
                                     start=(ki == 0), stop=(ki == nki - 1))
                for x in range(2):
                    h = 2 * hp + x
                    un = un_pool.tile([65, 512], F32, tag="un")
                    nc.vector.tensor_copy(un[:], pa[x][0:65, :])
                    attn_un.append((h, qt, un))
                    nc.sync.dma_start(sums16[h * 4 + qt:h * 4 + qt + 1, :],
                                      un[64:65, :])
        # ---- deferred softmax normalization ----
        nc.vector.reciprocal(sums16[:], sums16[:])
        rflat = sums_pool.tile([1, 16 * 512], F32, tag="rflat")
        nc.sync.dma_start(rflat[0:1, :].rearrange("p (a b) -> p a b", a=16), sums16[:])
        for h, qt, un in attn_un:
            i = h * 4 + qt
            rbc = norm_pool.tile([64, 512], F32, tag="rbc")
            nc.gpsimd.partition_broadcast(rbc[:], rflat[:, i * 512:(i + 1) * 512])
            fin = norm_pool.tile([64, 512], F32, tag="fin")
            nc.vector.tensor_mul(fin[:], un[0:64, :], rbc[:])
            nc.sync.dma_start(cc_in[h * 64:(h + 1) * 64, qt * 512:(qt + 1) * 512],
                              fin[:])

    # ---- AllGather + o_proj ----
    cc_out = dram.tile([H * D, S], F32)
    if TIMING_STUB:
        nc.sync.dma_start(cc_out[0:HQ * D, :], cc_in[:])
    else:
        nc.gpsimd.collective_compute(
            "AllGather", mybir.AluOpType.bypass,
            ins=[cc_in.opt()], outs=[cc_out.opt()],
            replica_groups=[list(range(NCORES))],
        )
    co3 = cc_out.rearrange("(ko p) s -> p ko s", p=128)
    with (
        tc.tile_pool(name="wo", bufs=1) as wo_pool,
        tc.tile_pool(name="attf", bufs=2) as attf_pool,
        tc.tile_pool(name="ops", bufs=2, space="PSUM") as o_ps,
    ):
        wo_sb = wo_pool.tile([128, KO, 256], F32R, tag="wo")
        nc.sync.dma_start(wo_sb[:], wo3.bitcast(F32R))
        for qt in range(ST):
            af = attf_pool.tile([128, KO, 512], F32R, tag="af")
            nc.sync.dma_start(af[:], co3[:, :, qt * 512:(qt + 1) * 512].bitcast(F32R))
            for ft in range(2):
                po = o_ps.tile([128, 512], F32, tag="po")
                for ko in range(KO):
                    nc.tensor.matmul(po[:], wo_sb[:, ko, ft * 128:(ft + 1) * 128],
                                     af[:, ko, :], start=(ko == 0),
                                     stop=(ko == KO - 1))
                ot = attf_pool.tile([128, 512], F32, tag="ot")
                nc.scalar.activation(ot[:], po[:], AF.Copy)
                nc.sync.dma_start(
                    outT[ft * 128:(ft + 1) * 128, qt * 512:(qt + 1) * 512], ot[:])
    es.close()


_CACHE = {}


def build_program():
    if "nc" in _CACHE:
        return _CACHE["nc"]
    nc = bacc.Bacc("TRN2", target_bir_lowering=False, debug=False,
                   enable_asserts=True, num_devices=NCORES)
    aps = {}
    aps["hiddenT"] = nc.dram_tensor("hiddenT", [HID, S], F32, kind="ExternalInput").ap()
    aps["wqkvT"] = nc.dram_tensor("wqkvT", [HID, (HQ + 2) * D], F32, kind="ExternalInput").ap()
    aps["woT"] = nc.dram_tensor("woT", [HID, HQ * D], F32, kind="ExternalInput").ap()
    aps["trimask"] = nc.dram_tensor("trimask", [128, 128], F32, kind="ExternalInput").ap()
    if DEVICE_TABLES:
        aps["posi"] = nc.dram_tensor("posi", [1, S], mybir.dt.int32, kind="ExternalInput").ap()
        aps["powers"] = nc.dram_tensor("powers", [D // 2, 1], F32, kind="ExternalInput").ap()
    else:
        aps["cos_t"] = nc.dram_tensor("cos_t", [128, S], F32, kind="ExternalInput").ap()
        aps["sin_t"] = nc.dram_tensor("sin_t", [128, S], F32, kind="ExternalInput").ap()
    aps["outT"] = nc.dram_tensor("outT", [HQ * D, S], F32, kind="ExternalOutput").ap()

    with tile.TileContext(nc) as tc:
        build_body(tc, aps)
    nc.compile()
    _CACHE["nc"] = nc
    return nc


def make_in_maps(hidden_states, position_ids, powers, Wq, Wk, Wv, Wo):
    hidden = np.asarray(hidden_states, np.float32).reshape(S, HID)
    hiddenT = np.ascontiguousarray(hidden.T)
    pos = np.asarray(position_ids, np.int32).reshape(1, S)
    pw = np.asarray(powers, np.float32).reshape(D // 2, 1)
    Wq = np.asarray(Wq, np.float32)
    Wk = np.asarray(Wk, np.float32)
    Wv = np.asarray(Wv, np.float32)
    Wo = np.asarray(Wo, np.float32)
    kl = np.arange(128)[:, None]
    ql = np.arange(128)[None, :]
    trimask = np.where(kl <= ql, 0.0, NEG).astype(np.float32)

    in_maps = []
    for c in range(NCORES):
        wqkv = np.concatenate([
            Wq[c * HQ * D:(c + 1) * HQ * D],          # [256, HID]
            Wk[c * D:(c + 1) * D],                    # [64, HID]
            Wv[c * D:(c + 1) * D],                    # [64, HID]
        ], axis=0)                                    # [384, HID]
        m = {
            "hiddenT": hiddenT,
            "wqkvT": np.ascontiguousarray(wqkv.T),
            "woT": np.ascontiguousarray(Wo[c * HQ * D:(c + 1) * HQ * D].T),
            "trimask": trimask,
        }
        if DEVICE_TABLES:
            m["posi"] = pos
            m["powers"] = pw
        else:
            inv_freq = (1.0 / BASE ** (1.0 / (1.0 + np.exp(-pw[:, 0])))).astype(np.float32)
            freqs = pos[0].astype(np.float32)[None, :] * inv_freq[:, None]  # [32, S]
            sin = np.sin(freqs).astype(np.float32)
            cos = np.cos(freqs).astype(np.float32)
            m["cos_t"] = np.ascontiguousarray(np.tile(cos, (4, 1)))
            m["sin_t"] = np.ascontiguousarray(
                np.concatenate([-sin, sin, -sin, sin], axis=0))
        in_maps.append(m)
    return in_maps


def run_spmd(nc, in_maps):
    m = nc.m
    nc.m = get_hw_module(nc.m)
    try:
        return bass_utils.run_bass_kernel_spmd(nc, in_maps, core_ids=list(range(NCORES)))
    finally:
        nc.m = m


def kernel(hidden_states, position_ids, powers, Wq, Wk, Wv, Wo):
    nc = build_program()
    in_maps = make_in_maps(hidden_states, position_ids, powers, Wq, Wk, Wv, Wo)
    res = run_spmd(nc, in_maps)
    outT_full = np.concatenate([res.results[c]["outT"] for c in range(NCORES)], axis=0)
    return np.ascontiguousarray(outT_full.T).reshape(1, S, HID).astype(np.float32)


if __name__ == "__main__":
    rng = np.random.default_rng(0)
    inputs = {
        "hidden_states": rng.standard_normal((1, S, HID), dtype=np.float32),
        "position_ids": np.broadcast_to(np.arange(S, dtype=np.int32), (1, S)),
        "powers": rng.standard_normal(D // 2).astype(np.float32),
        "Wq": (rng.standard_normal((H * D, HID)) * 0.02).astype(np.float32),
        "Wk": (rng.standard_normal((KV * D, HID)) * 0.02).astype(np.float32),
        "Wv": (rng.standard_normal((KV * D, HID)) * 0.02).astype(np.float32),
        "Wo": (rng.standard_normal((HID, H * D)) * 0.02).astype(np.float32),
    }
    out = kernel(**inputs)
    print("out", out.shape, out.dtype, np.abs(out).max())



# revision 6
# speedup vs baseline: 16134.3769x; 16134.3769x over previous
"""Trainium2 Bass kernel for LlamaAttention (B=1, S=2048, HID=2048, H=32, KV=8, D=64).

Sharding (8 cores): tensor-parallel over heads. Core c owns q-heads 4c..4c+3 and
kv-head c. Each core computes QKV projections for its heads, RoPE, causal
attention; attention outputs are normalized per 512-query tile, AllGathered in
bf16 chunks overlapped with the next tile's attention compute, and each core
computes 256 output features of o_proj (Wo row-sharded). Host concatenates the
8 column shards and transposes.

Matmuls run in bf16 (fp32 PSUM accumulation); softmax/rope arithmetic in fp32.
"""

import numpy as np
import ml_dtypes

import concourse.bass as bass
import concourse.mybir as mybir
import concourse.tile as tile
from concourse import bacc
from concourse import bass_utils
from concourse.bass_interp import get_hw_module
from concourse.masks import make_identity

S = 2048
HID = 2048
H = 32
KV = 8
D = 64
NCORES = 8
HQ = H // NCORES          # 4 q heads per core
BASE = 10000.0
F32 = mybir.dt.float32
BF16 = mybir.dt.bfloat16
AF = mybir.ActivationFunctionType
ST = S // 512             # 4 s/q tiles of 512
KO = HID // 128           # 16 contraction chunks
NEG = -1.0e30
HALF_PI = float(np.pi / 2)
LN_BASE = float(np.log(BASE))


def build_body(tc, aps):
    nc = tc.nc
    hiddenT = aps["hiddenT"]
    wqkvT = aps["wqkvT"]
    woT = aps["woT"]
    trimask = aps["trimask"]
    outT = aps["outT"]

    hT3 = hiddenT.rearrange("(ko p) s -> p ko s", p=128)
    wq3 = wqkvT.rearrange("(ko p) m -> p ko m", p=128)
    wo3 = woT.rearrange("(ko p) m -> p ko m", p=128)

    from contextlib import ExitStack
    es = ExitStack()
    const_pool = es.enter_context(tc.tile_pool(name="const", bufs=1))
    qkv_pool = es.enter_context(tc.tile_pool(name="qkvout", bufs=1))
    dram = es.enter_context(tc.tile_pool(name="dram", bufs=1, space="DRAM"))

    # ---- constants / weights ----
    mask_sb = const_pool.tile([128, 128], F32, tag="mask")
    nc.sync.dma_start(mask_sb[:], trimask[:])
    ident = const_pool.tile([64, 64], F32, tag="ident")
    make_identity(nc, ident[:])

    wq_sb = qkv_pool.tile([128, KO, 384], BF16, tag="wq")
    nc.sync.dma_start(wq_sb[:], wq3)
    wo_sb = qkv_pool.tile([128, KO, 256], BF16, tag="wo")
    nc.sync.dma_start(wo_sb[:], wo3)

    # ---- RoPE tables: cosT2/sinT2 [128, S] f32 ----
    cosT2 = const_pool.tile([128, S], F32, tag="cos")
    sinT2 = const_pool.tile([128, S], F32, tag="sin")
    posi = aps["posi"]
    powers = aps["powers"]
    with tc.tile_pool(name="tabtmp", bufs=1) as tab:
        # inv_freq = exp(-ln(BASE) * sigmoid(powers))  [32, 1]
        pw = tab.tile([32, 1], F32, tag="pw")
        nc.sync.dma_start(pw[:], powers[:])
        sg = tab.tile([32, 1], F32, tag="sg")
        nc.scalar.activation(sg[:], pw[:], AF.Sigmoid)
        invf = tab.tile([32, 1], F32, tag="invf")
        nc.scalar.activation(invf[:], sg[:], AF.Exp, scale=-LN_BASE)
        # signed replicate to [128, 1]: bands (-f, +f, -f, +f)
        invs = tab.tile([128, 1], F32, tag="invs")
        for b in range(4):
            nc.sync.dma_start(invs[b * 32:(b + 1) * 32, :], invf[:])
        for b in (0, 2):
            nc.scalar.activation(
                invs[b * 32:(b + 1) * 32, :], invs[b * 32:(b + 1) * 32, :],
                AF.Copy, scale=-1.0)
        # pos as f32 on one partition
        pos_i = tab.tile([1, S], mybir.dt.int32, tag="posi")
        nc.sync.dma_start(pos_i[:], posi[:])
        pos_f = tab.tile([1, S], F32, tag="posf")
        nc.vector.tensor_copy(pos_f[:], pos_i[:])
        pos_b = tab.tile([128, S], F32, tag="posb")
        nc.gpsimd.partition_broadcast(pos_b[:], pos_f[:])
        # freqs_signed = pos * invf_signed ; write into cosT2 temp
        nc.vector.tensor_scalar_mul(cosT2[:], pos_b[:], invs[:, 0:1])
        # range-reduce mod 2*pi so ACT Sin stays in its accurate domain:
        # x -= 2pi * round_to_int(x / 2pi)
        TWO_PI = 6.283185307179586
        u = tab.tile([128, S], F32, tag="u")
        nc.vector.tensor_scalar_mul(u[:], cosT2[:], 1.0 / TWO_PI)
        ui = tab.tile([128, S], mybir.dt.int32, tag="ui")
        nc.vector.tensor_copy(ui[:], u[:])
        nc.vector.tensor_copy(u[:], ui[:])
        nc.vector.tensor_scalar_mul(u[:], u[:], -TWO_PI)
        nc.vector.tensor_add(cosT2[:], cosT2[:], u[:])
        # sin/cos from signed freqs (sin odd, sin(x+pi/2)=cos(x))
        hpi = tab.tile([128, 1], F32, tag="hpi")
        nc.gpsimd.memset(hpi[:], HALF_PI)
        nc.scalar.activation(sinT2[:], cosT2[:], AF.Sin)
        nc.scalar.activation(cosT2[:], cosT2[:], AF.Sin, bias=hpi[:])

    # ---- persistent QKV outputs ----
    qT = [qkv_pool.tile([128, S], BF16, tag=f"qT{p}", name=f"qT{p}") for p in range(2)]
    kT2 = qkv_pool.tile([128, S], BF16, tag="kT2")
    vT = qkv_pool.tile([64, S], F32, tag="vT")
    vones = qkv_pool.tile([128, KO, 65], BF16, tag="vones")
    ones_f = const_pool.tile([128, 1], BF16, tag="onesf")
    nc.gpsimd.memset(ones_f[:], 1.0)
    nc.vector.tensor_copy(vones[:, :, 64:65], ones_f[:, 0:1, None].to_broadcast((128, KO, 1)))

    # ---- QKV projection + RoPE ----
    def rope(ps, dst, n_half, st, tmp_pool):
        """ps: psum [64*n_half, 512] raw. dst[:, st*512:...] = roped (bf16)."""
        sl = slice(st * 512, (st + 1) * 512)
        cs = cosT2[0:64 * n_half, sl]
        sn = sinT2[0:64 * n_half, sl]
        craw = tmp_pool.tile([64 * n_half, 512], F32, tag="craw")
        nc.scalar.activation(craw[:], ps, AF.Copy)
        sw = tmp_pool.tile([64 * n_half, 512], F32, tag="swap")
        for b in range(n_half):
            nc.sync.dma_start(sw[b * 64:b * 64 + 32, :], craw[b * 64 + 32:b * 64 + 64, :])
            nc.sync.dma_start(sw[b * 64 + 32:b * 64 + 64, :], craw[b * 64:b * 64 + 32, :])
        t1 = tmp_pool.tile([64 * n_half, 512], F32, tag="t1")
        nc.vector.tensor_mul(t1[:], ps, cs)
        nc.vector.tensor_mul(sw[:], sw[:], sn)
        nc.vector.tensor_add(dst[0:64 * n_half, sl], t1[:], sw[:])

    with (
        tc.tile_pool(name="hidd", bufs=2) as hidd_pool,
        tc.tile_pool(name="qkvps", bufs=2, space="PSUM") as qkv_ps,
        tc.tile_pool(name="kvps", bufs=2, space="PSUM") as kv_ps,
        tc.tile_pool(name="vtps", bufs=2, space="PSUM") as vt_ps,
        tc.tile_pool(name="ropetmp", bufs=3) as tmp_pool,
    ):
        for st in range(ST):
            ht = hidd_pool.tile([128, KO, 512], BF16, tag="ht")
            nc.sync.dma_start(ht[:], hT3[:, :, st * 512:(st + 1) * 512])
            for mt in range(2):  # q head pairs
                ps = qkv_ps.tile([128, 512], F32, tag="qps")
                for ko in range(KO):
                    nc.tensor.matmul(
                        ps[:], wq_sb[:, ko, mt * 128:(mt + 1) * 128],
                        ht[:, ko, :], start=(ko == 0), stop=(ko == KO - 1))
                rope(ps[:], qT[mt], 2, st, tmp_pool)
            # k+v combined (M=128: rows 0-63 = k, 64-127 = v)
            pskv = kv_ps.tile([128, 512], F32, tag="kvps")
            for ko in range(KO):
                nc.tensor.matmul(pskv[:], wq_sb[:, ko, 256:384], ht[:, ko, :],
                                 start=(ko == 0), stop=(ko == KO - 1))
            rope(pskv[0:64, :], kT2, 1, st, tmp_pool)
            nc.sync.dma_start(kT2[64:128, st * 512:(st + 1) * 512],
                              kT2[0:64, st * 512:(st + 1) * 512])
            nc.scalar.activation(vT[:, st * 512:(st + 1) * 512], pskv[64:128, :],
                                 AF.Copy)
            # transpose v chunks of this s-tile into vones [128, ki, 0:64]
            for kl in range(4):
                ki = st * 4 + kl
                pvt = vt_ps.tile([128, 64], F32, tag="vt")
                nc.tensor.transpose(pvt[:], vT[:, ki * 128:(ki + 1) * 128], ident[:])
                nc.vector.tensor_copy(vones[:, ki, 0:64], pvt[:])

    # ---- attention + overlapped AllGather + o_proj ----
    cc_in = [dram.tile([HQ * D, 512], BF16, tag=f"cc_in{qt}", name=f"cc_in{qt}")
             for qt in range(ST)]
    cc_out = [dram.tile([H * D, 512], BF16, tag=f"cc_out{qt}", name=f"cc_out{qt}")
              for qt in range(ST)]

    with (
        tc.tile_pool(name="sps", bufs=3, space="PSUM") as s_ps,
        tc.tile_pool(name="aps", bufs=2, space="PSUM") as a_ps,
        tc.tile_pool(name="ops", bufs=2, space="PSUM") as o_ps,
        tc.tile_pool(name="expp", bufs=34) as exp_pool,
        tc.tile_pool(name="norm", bufs=4) as norm_pool,
        tc.tile_pool(name="attf", bufs=2) as attf_pool,
        tc.tile_pool(name="oout", bufs=2) as oout_pool,
    ):
        def oproj(qt):
            co3 = cc_out[qt].rearrange("(ko p) s -> p ko s", p=128)
            af = attf_pool.tile([128, KO, 512], BF16, tag="af")
            nc.sync.dma_start(af[:], co3)
            for ft in range(2):
                po = o_ps.tile([128, 512], F32, tag="po")
                for ko in range(KO):
                    nc.tensor.matmul(po[:], wo_sb[:, ko, ft * 128:(ft + 1) * 128],
                                     af[:, ko, :], start=(ko == 0),
                                     stop=(ko == KO - 1))
                ot = oout_pool.tile([128, 512], F32, tag="ot")
                nc.vector.tensor_copy(ot[:], po[:])
                nc.sync.dma_start(
                    outT[ft * 128:(ft + 1) * 128, qt * 512:(qt + 1) * 512], ot[:])

        for qt in range(ST):
            nki = 4 * qt + 4
            for hp in range(2):
                pa = [a_ps.tile([65, 512], F32, tag="pattn",
                                name=f"pattn{qt}_{hp}_{i}") for i in range(2)]
                staged = []
                for ki in range(nki):
                    for x in range(2):  # head 2hp+x
                        m = ki - 4 * qt
                        lo = max(0, m) * 128
                        pss = s_ps.tile([128, 512], F32, tag="ps_s")
                        nc.tensor.matmul(
                            pss[:, lo:512],
                            kT2[x * 64:(x + 1) * 64, ki * 128:(ki + 1) * 128],
                            qT[hp][x * 64:(x + 1) * 64, qt * 512 + lo:(qt + 1) * 512],
                            start=True, stop=True)
                        if m >= 0:  # diagonal block: apply triangular causal mask
                            nc.vector.tensor_add(
                                pss[:, m * 128:(m + 1) * 128],
                                pss[:, m * 128:(m + 1) * 128], mask_sb[:])
                        et = exp_pool.tile([128, 512], BF16, tag="expt")
                        nc.scalar.activation(et[:, lo:512], pss[:, lo:512],
                                             AF.Exp, scale=0.125)
                        staged.append((ki, x, lo, et))
                # second pass: attn@v matmuls read staged SBUF exp tiles, so the
                # PE never blocks on ACT latency mid-stream
                for ki, x, lo, et in staged:
                    nc.tensor.matmul(pa[x][:, lo:512], vones[:, ki, :],
                                     et[:, lo:512],
                                     start=(ki == 0), stop=(ki == nki - 1))
                # normalize the 2 finished heads: r = 1/sums (row 64), then
                # per-query scale and bf16 cast into the collective chunk
                r2 = norm_pool.tile([1, 2 * 512], F32, tag="r2")
                for x in range(2):
                    nc.vector.tensor_copy(r2[0:1, x * 512:(x + 1) * 512],
                                          pa[x][64:65, :])
                nc.vector.reciprocal(r2[:], r2[:])
                for x in range(2):
                    rbc = norm_pool.tile([64, 512], F32, tag="rbc")
                    nc.gpsimd.partition_broadcast(rbc[:], r2[:, x * 512:(x + 1) * 512])
                    fin = norm_pool.tile([64, 512], BF16, tag="fin")
                    nc.vector.tensor_mul(fin[:], pa[x][0:64, :], rbc[:])
                    h = 2 * hp + x
                    nc.sync.dma_start(cc_in[qt][h * 64:(h + 1) * 64, :], fin[:])
            nc.gpsimd.collective_compute(
                "AllGather", mybir.AluOpType.bypass,
                ins=[cc_in[qt].opt()], outs=[cc_out[qt].opt()],
                replica_groups=[list(range(NCORES))],
            )
            if qt >= 1:
                oproj(qt - 1)
        oproj(ST - 1)
    es.close()


_CACHE = {}


def build_program():
    if "nc" in _CACHE:
        return _CACHE["nc"]
    nc = bacc.Bacc("TRN2", target_bir_lowering=False, debug=False,
                   enable_asserts=True, num_devices=NCORES)
    aps = {}
    aps["hiddenT"] = nc.dram_tensor("hiddenT", [HID, S], BF16, kind="ExternalInput").ap()
    aps["wqkvT"] = nc.dram_tensor("wqkvT", [HID, (HQ + 2) * D], BF16, kind="ExternalInput").ap()
    aps["woT"] = nc.dram_tensor("woT", [HID, HQ * D], BF16, kind="ExternalInput").ap()
    aps["trimask"] = nc.dram_tensor("trimask", [128, 128], F32, kind="ExternalInput").ap()
    aps["posi"] = nc.dram_tensor("posi", [1, S], mybir.dt.int32, kind="ExternalInput").ap()
    aps["powers"] = nc.dram_tensor("powers", [D // 2, 1], F32, kind="ExternalInput").ap()
    aps["outT"] = nc.dram_tensor("outT", [HQ * D, S], F32, kind="ExternalOutput").ap()

    with tile.TileContext(nc) as tc:
        build_body(tc, aps)
    nc.compile()
    _CACHE["nc"] = nc
    return nc


def make_in_maps(hidden_states, position_ids, powers, Wq, Wk, Wv, Wo):
    bf16 = ml_dtypes.bfloat16
    hidden = np.asarray(hidden_states, np.float32).reshape(S, HID)
    hiddenT = np.ascontiguousarray(hidden.T).astype(bf16)
    pos = np.asarray(position_ids, np.int32).reshape(1, S)
    pw = np.asarray(powers, np.float32).reshape(D // 2, 1)
    Wq = np.asarray(Wq, np.float32)
    Wk = np.asarray(Wk, np.float32)
    Wv = np.asarray(Wv, np.float32)
    Wo = np.asarray(Wo, np.float32)
    kl = np.arange(128)[:, None]
    ql = np.arange(128)[None, :]
    trimask = np.where(kl <= ql, 0.0, NEG).astype(np.float32)

    in_maps = []
    for c in range(NCORES):
        wqkv = np.concatenate([
            Wq[c * HQ * D:(c + 1) * HQ * D],          # [256, HID]
            Wk[c * D:(c + 1) * D],                    # [64, HID]
            Wv[c * D:(c + 1) * D],                    # [64, HID]
        ], axis=0)                                    # [384, HID]
        m = {
            "hiddenT": hiddenT,
            "wqkvT": np.ascontiguousarray(wqkv.T).astype(bf16),
            "woT": np.ascontiguousarray(Wo[c * HQ * D:(c + 1) * HQ * D].T).astype(bf16),
            "trimask": trimask,
            "posi": pos,
            "powers": pw,
        }
        in_maps.append(m)
    return in_maps


def run_spmd(nc, in_maps, **kwargs):
    m = nc.m
    nc.m = get_hw_module(nc.m)
    try:
        return bass_utils.run_bass_kernel_spmd(
            nc, in_maps, core_ids=list(range(NCORES)), **kwargs)
    finally:
        nc.m = m


def kernel(hidden_states, position_ids, powers, Wq, Wk, Wv, Wo):
    nc = build_program()
    in_maps = make_in_maps(hidden_states, position_ids, powers, Wq, Wk, Wv, Wo)
    res = run_spmd(nc, in_maps)
    outT_full = np.concatenate([res.results[c]["outT"] for c in range(NCORES)], axis=0)
    return np.ascontiguousarray(outT_full.T).reshape(1, S, HID).astype(np.float32)


if __name__ == "__main__":
    rng = np.random.default_rng(0)
    inputs = {
        "hidden_states": rng.standard_normal((1, S, HID), dtype=np.float32),
        "position_ids": np.broadcast_to(np.arange(S, dtype=np.int32), (1, S)),
        "powers": rng.standard_normal(D // 2).astype(np.float32),
        "Wq": (rng.standard_normal((H * D, HID)) * 0.02).astype(np.float32),
        "Wk": (rng.standard_normal((KV * D, HID)) * 0.02).astype(np.float32),
        "Wv": (rng.standard_normal((KV * D, HID)) * 0.02).astype(np.float32),
        "Wo": (rng.standard_normal((HID, H * D)) * 0.02).astype(np.float32),
    }
    out = kernel(**inputs)
    print("out", out.shape, out.dtype, np.abs(out).max())


# revision 11
# speedup vs baseline: 18531.9022x; 1.1486x over previous
"""Trainium2 Bass kernel for LlamaAttention (B=1, S=2048, HID=2048, H=32, KV=8, D=64).

Sharding (8 cores): tensor-parallel over heads. Core c owns q-heads 4c..4c+3 and
kv-head c. Each core computes QKV projections for its heads, RoPE, causal
attention; attention outputs are normalized per 512-query tile, AllGathered in
bf16 chunks overlapped with the next tile's attention compute, and each core
computes 256 output features of o_proj (Wo row-sharded). Host concatenates the
8 column shards and transposes.

Matmuls run in bf16 (fp32 PSUM accumulation); softmax/rope arithmetic in fp32.
"""

import numpy as np
import ml_dtypes

import concourse.bass as bass
import concourse.mybir as mybir
import concourse.tile as tile
from concourse import bacc
from concourse import bass_utils
from concourse.bass_interp import get_hw_module
from concourse.masks import make_identity

S = 2048
HID = 2048
H = 32
KV = 8
D = 64
NCORES = 8
HQ = H // NCORES          # 4 q heads per core
BASE = 10000.0
F32 = mybir.dt.float32
BF16 = mybir.dt.bfloat16
AF = mybir.ActivationFunctionType
ST = S // 512             # 4 s/q tiles of 512
KO = HID // 128           # 16 contraction chunks
NEG = -1.0e30
HALF_PI = float(np.pi / 2)
LN_BASE = float(np.log(BASE))


def build_body(tc, aps):
    nc = tc.nc
    hiddenT = aps["hiddenT"]
    wqkvT = aps["wqkvT"]
    woT = aps["woT"]
    trimask = aps["trimask"]
    outT = aps["outT"]

    hT3 = hiddenT.rearrange("(ko p) s -> p ko s", p=128)
    wq3 = wqkvT.rearrange("(ko p) m -> p ko m", p=128)
    wo3 = woT.rearrange("(ko p) m -> p ko m", p=128)

    from contextlib import ExitStack
    es = ExitStack()
    const_pool = es.enter_context(tc.tile_pool(name="const", bufs=1))
    qkv_pool = es.enter_context(tc.tile_pool(name="qkvout", bufs=1))
    dram = es.enter_context(tc.tile_pool(name="dram", bufs=1, space="DRAM"))

    # ---- constants / weights ----
    mask_sb = const_pool.tile([128, 128], F32, tag="mask")
    nc.sync.dma_start(mask_sb[:], trimask[:])
    ident = const_pool.tile([64, 64], F32, tag="ident")
    make_identity(nc, ident[:])

    wq_sb = qkv_pool.tile([128, KO, 384], BF16, tag="wq")
    nc.sync.dma_start(wq_sb[:], wq3)
    wo_sb = qkv_pool.tile([128, KO, 256], BF16, tag="wo")
    nc.sync.dma_start(wo_sb[:], wo3)

    # ---- RoPE tables: cosT2/sinT2 [128, S] f32 ----
    cosT2 = const_pool.tile([128, S], F32, tag="cos")
    sinT2 = const_pool.tile([128, S], F32, tag="sin")
    posi = aps["posi"]
    powers = aps["powers"]
    with tc.tile_pool(name="tabtmp", bufs=1) as tab:
        # inv_freq = exp(-ln(BASE) * sigmoid(powers))  [32, 1]
        pw = tab.tile([32, 1], F32, tag="pw")
        nc.sync.dma_start(pw[:], powers[:])
        sg = tab.tile([32, 1], F32, tag="sg")
        nc.scalar.activation(sg[:], pw[:], AF.Sigmoid)
        invf = tab.tile([32, 1], F32, tag="invf")
        nc.scalar.activation(invf[:], sg[:], AF.Exp, scale=-LN_BASE)
        # signed replicate to [128, 1]: bands (-f, +f, -f, +f)
        invs = tab.tile([128, 1], F32, tag="invs")
        for b in range(4):
            nc.sync.dma_start(invs[b * 32:(b + 1) * 32, :], invf[:])
        for b in (0, 2):
            nc.scalar.activation(
                invs[b * 32:(b + 1) * 32, :], invs[b * 32:(b + 1) * 32, :],
                AF.Copy, scale=-1.0)
        # pos as f32 on one partition
        pos_i = tab.tile([1, S], mybir.dt.int32, tag="posi")
        nc.sync.dma_start(pos_i[:], posi[:])
        pos_f = tab.tile([1, S], F32, tag="posf")
        nc.vector.tensor_copy(pos_f[:], pos_i[:])
        pos_b = tab.tile([128, S], F32, tag="posb")
        nc.gpsimd.partition_broadcast(pos_b[:], pos_f[:])
        # freqs_signed = pos * invf_signed ; write into cosT2 temp
        nc.vector.tensor_scalar_mul(cosT2[:], pos_b[:], invs[:, 0:1])
        # range-reduce mod 2*pi so ACT Sin stays in its accurate domain:
        # x -= 2pi * round_to_int(x / 2pi)
        TWO_PI = 6.283185307179586
        u = tab.tile([128, S], F32, tag="u")
        nc.vector.tensor_scalar_mul(u[:], cosT2[:], 1.0 / TWO_PI)
        ui = tab.tile([128, S], mybir.dt.int32, tag="ui")
        nc.vector.tensor_copy(ui[:], u[:])
        nc.vector.tensor_copy(u[:], ui[:])
        nc.vector.tensor_scalar_mul(u[:], u[:], -TWO_PI)
        nc.vector.tensor_add(cosT2[:], cosT2[:], u[:])
        # sin/cos from signed freqs (sin odd, sin(x+pi/2)=cos(x))
        hpi = tab.tile([128, 1], F32, tag="hpi")
        nc.gpsimd.memset(hpi[:], HALF_PI)
        nc.scalar.activation(sinT2[:], cosT2[:], AF.Sin)
        nc.scalar.activation(cosT2[:], cosT2[:], AF.Sin, bias=hpi[:])

    # ---- persistent QKV outputs ----
    qT = [qkv_pool.tile([128, S], BF16, tag=f"qT{p}", name=f"qT{p}") for p in range(2)]
    kT2 = qkv_pool.tile([128, S], BF16, tag="kT2")
    vT = qkv_pool.tile([64, S], F32, tag="vT")
    vones = qkv_pool.tile([128, KO, 65], BF16, tag="vones")
    ones_f = const_pool.tile([128, 1], BF16, tag="onesf")
    nc.gpsimd.memset(ones_f[:], 1.0)
    nc.vector.tensor_copy(vones[:, :, 64:65], ones_f[:, 0:1, None].to_broadcast((128, KO, 1)))

    # ---- QKV projection + RoPE ----
    def rope(ps, dst, n_half, st, tmp_pool):
        """ps: psum [64*n_half, 512] raw. dst[:, st*512:...] = roped (bf16)."""
        sl = slice(st * 512, (st + 1) * 512)
        cs = cosT2[0:64 * n_half, sl]
        sn = sinT2[0:64 * n_half, sl]
        craw = tmp_pool.tile([64 * n_half, 512], F32, tag="craw")
        nc.scalar.activation(craw[:], ps, AF.Copy)
        sw = tmp_pool.tile([64 * n_half, 512], F32, tag="swap")
        for b in range(n_half):
            nc.sync.dma_start(sw[b * 64:b * 64 + 32, :], craw[b * 64 + 32:b * 64 + 64, :])
            nc.sync.dma_start(sw[b * 64 + 32:b * 64 + 64, :], craw[b * 64:b * 64 + 32, :])
        t1 = tmp_pool.tile([64 * n_half, 512], F32, tag="t1")
        nc.vector.tensor_mul(t1[:], ps, cs)
        nc.vector.tensor_mul(sw[:], sw[:], sn)
        nc.vector.tensor_add(dst[0:64 * n_half, sl], t1[:], sw[:])

    with (
        tc.tile_pool(name="hidd", bufs=2) as hidd_pool,
        tc.tile_pool(name="qkvps", bufs=3, space="PSUM") as qkv_ps,
        tc.tile_pool(name="kvps", bufs=2, space="PSUM") as kv_ps,
        tc.tile_pool(name="vtps", bufs=2, space="PSUM") as vt_ps,
        tc.tile_pool(name="ropetmp", bufs=3) as tmp_pool,
    ):
        for st in range(ST):
            ht = hidd_pool.tile([128, KO, 512], BF16, tag="ht")
            nc.sync.dma_start(ht[:], hT3[:, :, st * 512:(st + 1) * 512])
            for mt in range(2):  # q head pairs
                ps = qkv_ps.tile([128, 512], F32, tag="qps")
                for ko in range(KO):
                    nc.tensor.matmul(
                        ps[:], wq_sb[:, ko, mt * 128:(mt + 1) * 128],
                        ht[:, ko, :], start=(ko == 0), stop=(ko == KO - 1))
                rope(ps[:], qT[mt], 2, st, tmp_pool)
            # k+v combined (M=128: rows 0-63 = k, 64-127 = v)
            pskv = kv_ps.tile([128, 512], F32, tag="kvps")
            for ko in range(KO):
                nc.tensor.matmul(pskv[:], wq_sb[:, ko, 256:384], ht[:, ko, :],
                                 start=(ko == 0), stop=(ko == KO - 1))
            rope(pskv[0:64, :], kT2, 1, st, tmp_pool)
            nc.sync.dma_start(kT2[64:128, st * 512:(st + 1) * 512],
                              kT2[0:64, st * 512:(st + 1) * 512])
            nc.scalar.activation(vT[:, st * 512:(st + 1) * 512], pskv[64:128, :],
                                 AF.Copy)
            # transpose v chunks of this s-tile into vones [128, ki, 0:64]
            for kl in range(4):
                ki = st * 4 + kl
                pvt = vt_ps.tile([128, 64], F32, tag="vt")
                nc.tensor.transpose(pvt[:], vT[:, ki * 128:(ki + 1) * 128], ident[:])
                nc.vector.tensor_copy(vones[:, ki, 0:64], pvt[:])

    # ---- attention + overlapped AllGather + o_proj ----
    cc_in = [dram.tile([HQ * D, 512], BF16, tag=f"cc_in{qt}", name=f"cc_in{qt}")
             for qt in range(ST)]
    cc_out = [dram.tile([H * D, 512], BF16, tag=f"cc_out{qt}", name=f"cc_out{qt}")
              for qt in range(ST)]

    with (
        tc.tile_pool(name="sps", bufs=3, space="PSUM") as s_ps,
        tc.tile_pool(name="aps", bufs=2, space="PSUM") as a_ps,
        tc.tile_pool(name="ops", bufs=2, space="PSUM") as o_ps,
        tc.tile_pool(name="expp", bufs=34) as exp_pool,
        tc.tile_pool(name="norm", bufs=4) as norm_pool,
        tc.tile_pool(name="attf", bufs=2) as attf_pool,
        tc.tile_pool(name="oout", bufs=2) as oout_pool,
    ):
        def oproj(qt):
            co3 = cc_out[qt].rearrange("(ko p) s -> p ko s", p=128)
            af = attf_pool.tile([128, KO, 512], BF16, tag="af")
            nc.sync.dma_start(af[:], co3)
            for ft in range(2):
                po = o_ps.tile([128, 512], F32, tag="po")
                for ko in range(KO):
                    nc.tensor.matmul(po[:], wo_sb[:, ko, ft * 128:(ft + 1) * 128],
                                     af[:, ko, :], start=(ko == 0),
                                     stop=(ko == KO - 1))
                ot = oout_pool.tile([128, 512], F32, tag="ot")
                nc.vector.tensor_copy(ot[:], po[:])
                nc.sync.dma_start(
                    outT[ft * 128:(ft + 1) * 128, qt * 512:(qt + 1) * 512], ot[:])

        for qt in range(ST):
            nki = 4 * qt + 4
            for hp in range(2):
                pa = [a_ps.tile([65, 512], F32, tag="pattn",
                                name=f"pattn{qt}_{hp}_{i}") for i in range(2)]
                staged = []
                for ki in range(nki):
                    for x in range(2):  # head 2hp+x
                        m = ki - 4 * qt
                        lo = max(0, m) * 128
                        pss = s_ps.tile([128, 512], F32, tag="ps_s")
                        nc.tensor.matmul(
                            pss[:, lo:512],
                            kT2[x * 64:(x + 1) * 64, ki * 128:(ki + 1) * 128],
                            qT[hp][x * 64:(x + 1) * 64, qt * 512 + lo:(qt + 1) * 512],
                            start=True, stop=True)
                        if m >= 0:  # diagonal block: apply triangular causal mask
                            nc.vector.tensor_add(
                                pss[:, m * 128:(m + 1) * 128],
                                pss[:, m * 128:(m + 1) * 128], mask_sb[:])
                        et = exp_pool.tile([128, 512], BF16, tag="expt")
                        nc.scalar.activation(et[:, lo:512], pss[:, lo:512],
                                             AF.Exp, scale=0.125)
                        staged.append((ki, x, lo, et))
                # second pass: attn@v matmuls read staged SBUF exp tiles, so the
                # PE never blocks on ACT latency mid-stream
                for ki, x, lo, et in staged:
                    nc.tensor.matmul(pa[x][:, lo:512], vones[:, ki, :],
                                     et[:, lo:512],
                                     start=(ki == 0), stop=(ki == nki - 1))
                # normalize the 2 finished heads: r = 1/sums (row 64), then
                # per-query scale and bf16 cast into the collective chunk
                r2 = norm_pool.tile([1, 2 * 512], F32, tag="r2")
                for x in range(2):
                    nc.vector.tensor_copy(r2[0:1, x * 512:(x + 1) * 512],
                                          pa[x][64:65, :])
                # DVE reciprocal iterates per free-dim column: reshape the
                # 1024 sums to [128, 8] so the lanes do the work in parallel
                rp = norm_pool.tile([128, 8], F32, tag="rp")
                nc.sync.dma_start(rp[:], r2[0:1, :].rearrange(
                    "q (p f) -> q p f", p=128))
                nc.vector.reciprocal_approx_fast(rp[:], rp[:])
                nc.sync.dma_start(r2[0:1, :].rearrange(
                    "q (p f) -> q p f", p=128), rp[:])
                for x in range(2):
                    rbc = norm_pool.tile([64, 512], F32, tag="rbc")
                    nc.gpsimd.partition_broadcast(rbc[:], r2[:, x * 512:(x + 1) * 512])
                    fin = norm_pool.tile([64, 512], BF16, tag="fin")
                    nc.vector.tensor_mul(fin[:], pa[x][0:64, :], rbc[:])
                    h = 2 * hp + x
                    nc.sync.dma_start(cc_in[qt][h * 64:(h + 1) * 64, :], fin[:])
            nc.gpsimd.collective_compute(
                "AllGather", mybir.AluOpType.bypass,
                ins=[cc_in[qt].opt()], outs=[cc_out[qt].opt()],
                replica_groups=[list(range(NCORES))],
            )
            if qt >= 1:
                oproj(qt - 1)
        oproj(ST - 1)
    es.close()


_CACHE = {}


def build_program():
    if "nc" in _CACHE:
        return _CACHE["nc"]
    nc = bacc.Bacc("TRN2", target_bir_lowering=False, debug=False,
                   enable_asserts=True, num_devices=NCORES)
    aps = {}
    aps["hiddenT"] = nc.dram_tensor("hiddenT", [HID, S], BF16, kind="ExternalInput").ap()
    aps["wqkvT"] = nc.dram_tensor("wqkvT", [HID, (HQ + 2) * D], BF16, kind="ExternalInput").ap()
    aps["woT"] = nc.dram_tensor("woT", [HID, HQ * D], BF16, kind="ExternalInput").ap()
    aps["trimask"] = nc.dram_tensor("trimask", [128, 128], F32, kind="ExternalInput").ap()
    aps["posi"] = nc.dram_tensor("posi", [1, S], mybir.dt.int32, kind="ExternalInput").ap()
    aps["powers"] = nc.dram_tensor("powers", [D // 2, 1], F32, kind="ExternalInput").ap()
    aps["outT"] = nc.dram_tensor("outT", [HQ * D, S], F32, kind="ExternalOutput").ap()

    with tile.TileContext(nc) as tc:
        build_body(tc, aps)
    nc.compile()
    _CACHE["nc"] = nc
    return nc


def make_in_maps(hidden_states, position_ids, powers, Wq, Wk, Wv, Wo):
    bf16 = ml_dtypes.bfloat16
    hidden = np.asarray(hidden_states, np.float32).reshape(S, HID)
    hiddenT = np.ascontiguousarray(hidden.T).astype(bf16)
    pos = np.asarray(position_ids, np.int32).reshape(1, S)
    pw = np.asarray(powers, np.float32).reshape(D // 2, 1)
    Wq = np.asarray(Wq, np.float32)
    Wk = np.asarray(Wk, np.float32)
    Wv = np.asarray(Wv, np.float32)
    Wo = np.asarray(Wo, np.float32)
    kl = np.arange(128)[:, None]
    ql = np.arange(128)[None, :]
    trimask = np.where(kl <= ql, 0.0, NEG).astype(np.float32)

    in_maps = []
    for c in range(NCORES):
        wqkv = np.concatenate([
            Wq[c * HQ * D:(c + 1) * HQ * D],          # [256, HID]
            Wk[c * D:(c + 1) * D],                    # [64, HID]
            Wv[c * D:(c + 1) * D],                    # [64, HID]
        ], axis=0)                                    # [384, HID]
        m = {
            "hiddenT": hiddenT,
            "wqkvT": np.ascontiguousarray(wqkv.T).astype(bf16),
            "woT": np.ascontiguousarray(Wo[c * HQ * D:(c + 1) * HQ * D].T).astype(bf16),
            "trimask": trimask,
            "posi": pos,
            "powers": pw,
        }
        in_maps.append(m)
    return in_maps


def run_spmd(nc, in_maps, **kwargs):
    m = nc.m
    nc.m = get_hw_module(nc.m)
    try:
        return bass_utils.run_bass_kernel_spmd(
            nc, in_maps, core_ids=list(range(NCORES)), **kwargs)
    finally:
        nc.m = m


def kernel(hidden_states, position_ids, powers, Wq, Wk, Wv, Wo):
    nc = build_program()
    in_maps = make_in_maps(hidden_states, position_ids, powers, Wq, Wk, Wv, Wo)
    res = run_spmd(nc, in_maps)
    outT_full = np.concatenate([res.results[c]["outT"] for c in range(NCORES)], axis=0)
    return np.ascontiguousarray(outT_full.T).reshape(1, S, HID).astype(np.float32)


if __name__ == "__main__":
    rng = np.random.default_rng(0)
    inputs = {
        "hidden_states": rng.standard_normal((1, S, HID), dtype=np.float32),
        "position_ids": np.broadcast_to(np.arange(S, dtype=np.int32), (1, S)),
        "powers": rng.standard_normal(D // 2).astype(np.float32),
        "Wq": (rng.standard_normal((H * D, HID)) * 0.02).astype(np.float32),
        "Wk": (rng.standard_normal((KV * D, HID)) * 0.02).astype(np.float32),
        "Wv": (rng.standard_normal((KV * D, HID)) * 0.02).astype(np.float32),
        "Wo": (rng.standard_normal((HID, H * D)) * 0.02).astype(np.float32),
    }
    out = kernel(**inputs)
    print("out", out.shape, out.dtype, np.abs(out).max())
